# revision 1
# baseline (speedup 1.0000x reference)
"""ComplEx decoder edge scoring on 8 Trainium2 NeuronCores.

score[e] = sum_d Re( s_e * r_e * o_e )  for complex embeddings
         = sum_d [ r_re*(s_re*o_re - s_im*o_im) - r_im*(s_re*o_im + s_im*o_re) ]

Strategy (pure edge parallelism, hint-compliant):
  - Edges dealt round-robin to the 8 cores within each (src_chunk, dst_chunk)
    bin; node/relation tables replicated per core. 32768-row node chunks keep
    chunk-local row ids inside the int16 index format of the SWDGE dma_gather
    instruction. Bin capacities are shared across cores (rounded up to 256)
    so a single SPMD NEFF serves all 8 cores.
  - Device loop per 4096-edge batch: three dma_gather streams pull per-edge
    rows (s 512B from [x_re|x_im], o 512B from [x_re|-x_im], and relation
    rows 1KB from a 100x100 paired table [rc_t1|rc_t2] so one descriptor
    serves two edges), then DVE elementwise complex product + weighting +
    free-dim reduction produces one f32 score per edge.
  - Host side only slices/sorts indices and un-permutes the scores.

Performance note: throughput is pinned by the SWDGE Q7 descriptor-generation
rate (~8 ns/descriptor aggregate, measured; SDMA engines sit ~75% idle), so
the design minimizes descriptor count: 2.5 descriptors per edge.
"""

import numpy as np
from contextlib import ExitStack

import concourse.tile as tile
from concourse import bacc, mybir
from concourse.bass_utils import run_bass_kernel_spmd

N_CORES = 8
CHUNK = 32768          # node-table chunk rows (int16 index limit)
BATCH = 4096           # edges per compute batch
P = 128                # SBUF partitions
D2 = 128               # concat feature dim (2 * D)

_cache = {}
_last_results = None


def _build(n_nodes, n_rel, e_pad, bin_caps, n_chunks):
    """Compile the SPMD kernel for the given static layout."""
    f32 = mybir.dt.float32
    i16 = mybir.dt.int16

    # batch boundaries and per-batch gather segments (cut at bin boundaries)
    bin_starts = np.concatenate([[0], np.cumsum(bin_caps)])
    batches = []
    pos = 0
    while pos < e_pad:
        blen = min(BATCH, e_pad - pos)
        # segments: [pos, pos+blen) cut at bin boundaries
        segs = []
        for b in range(len(bin_caps)):
            lo = max(pos, int(bin_starts[b]))
            hi = min(pos + blen, int(bin_starts[b + 1]))
            if lo < hi:
                segs.append((lo, hi - lo, b // n_chunks, b % n_chunks))
        batches.append((pos, blen, segs))
        pos += blen

    nc = bacc.Bacc("TRN2")
    xcat = nc.dram_tensor("xcat", [n_nodes, D2], f32, kind="ExternalInput")
    xneg = nc.dram_tensor("xneg", [n_nodes, D2], f32, kind="ExternalInput")
    rpair = nc.dram_tensor("rpair", [n_rel * n_rel, 2 * D2], f32, kind="ExternalInput")
    idx_s = nc.dram_tensor("idx_s", [P, e_pad // 16], i16, kind="ExternalInput")
    idx_o = nc.dram_tensor("idx_o", [P, e_pad // 16], i16, kind="ExternalInput")
    idx_r = nc.dram_tensor("idx_r", [P, e_pad // 32], i16, kind="ExternalInput")
    out = nc.dram_tensor("out", [P, e_pad // 128], f32, kind="ExternalOutput")

    with ExitStack() as ctx:
        tc = ctx.enter_context(tile.TileContext(nc))
        ipool = ctx.enter_context(tc.tile_pool(name="idx", bufs=3))
        gpool = ctx.enter_context(tc.tile_pool(name="gath", bufs=2))
        tpool = ctx.enter_context(tc.tile_pool(name="tmp", bufs=1))
        spool = ctx.enter_context(tc.tile_pool(name="scores", bufs=1))

        scores = spool.tile([P, e_pad // 128], f32)

        for pos, blen, segs in batches:
            g = blen // 128
            cols = blen // 16
            it_s = ipool.tile([P, cols], i16, tag="it_s")
            it_o = ipool.tile([P, cols], i16, tag="it_o")
            it_r = ipool.tile([P, cols // 2], i16, tag="it_r")
            nc.sync.dma_start(it_s[:], idx_s[:, pos // 16 : pos // 16 + cols])
            nc.sync.dma_start(it_o[:], idx_o[:, pos // 16 : pos // 16 + cols])
            nc.sync.dma_start(it_r[:], idx_r[:, pos // 32 : pos // 32 + cols // 2])

            S = gpool.tile([P, g, D2], f32, tag="S")
            O = gpool.tile([P, g, D2], f32, tag="O")
            RC = gpool.tile([P, g, D2], f32, tag="RC")
            for (L, n, cs, co) in segs:
                g0 = (L - pos) // 128
                c0 = (L - pos) // 16
                sl_s = xcat[cs * CHUNK : min((cs + 1) * CHUNK, n_nodes), :]
                sl_o = xneg[co * CHUNK : min((co + 1) * CHUNK, n_nodes), :]
                nc.gpsimd.dma_gather(
                    S[:, g0 : g0 + n // 128, :], sl_s,
                    it_s[:, c0 : c0 + n // 16], n, n, D2,
                    single_packet=False,
                )
                nc.gpsimd.dma_gather(
                    O[:, g0 : g0 + n // 128, :], sl_o,
                    it_o[:, c0 : c0 + n // 16], n, n, D2,
                    single_packet=False,
                )
            rc_pairview = RC[:, :g, :].rearrange(
                "p (h two) d -> p h (two d)", two=2
            )
            nc.gpsimd.dma_gather(
                rc_pairview, rpair[:, :], it_r[:, : cols // 2],
                blen // 2, blen // 2, 2 * D2, single_packet=False,
            )

            # PQ[:, :, 0:128]   = S * O           -> [s_re*o_re | -s_im*o_im]
            # PQ[:, :, 128:192] = S_hi * O_lo     ->  s_im*o_re
            # PQ[:, :, 192:256] = S_lo * O_hi     -> -s_re*o_im
            PQ = tpool.tile([P, g, 256], f32, tag="PQ")
            nc.vector.tensor_tensor(
                out=PQ[:, :, 0:128], in0=S[:, :, :], in1=O[:, :, :],
                op=mybir.AluOpType.mult,
            )
            nc.vector.tensor_tensor(
                out=PQ[:, :, 128:192], in0=S[:, :, 64:128], in1=O[:, :, 0:64],
                op=mybir.AluOpType.mult,
            )
            nc.vector.tensor_tensor(
                out=PQ[:, :, 192:256], in0=S[:, :, 0:64], in1=O[:, :, 64:128],
                op=mybir.AluOpType.mult,
            )
            # A = s_re*o_re - s_im*o_im = PQ[0:64] + PQ[64:128]      (add)
            # B = s_im*o_re + s_re*o_im = PQ[128:192] - PQ[192:256]  (subtract)
            AB = tpool.tile([P, g, D2], f32, tag="AB")
            nc.vector.tensor_tensor(
                out=AB[:, :, 0:64], in0=PQ[:, :, 0:64], in1=PQ[:, :, 64:128],
                op=mybir.AluOpType.add,
            )
            nc.vector.tensor_tensor(
                out=AB[:, :, 64:128], in0=PQ[:, :, 128:192], in1=PQ[:, :, 192:256],
                op=mybir.AluOpType.subtract,
            )
            # W = AB * RC, rc rows = [r_re | -r_im]; overwrite PQ's P-half as scratch
            nc.vector.tensor_tensor(
                out=PQ[:, :, 0:128], in0=AB[:, :, :], in1=RC[:, :g, :],
                op=mybir.AluOpType.mult,
            )
            nc.vector.tensor_reduce(
                out=scores[:, pos // 128 : pos // 128 + g],
                in_=PQ[:, :, 0:128],
                axis=mybir.AxisListType.X,
                op=mybir.AluOpType.add,
            )

        nc.sync.dma_start(out[:], scores[:])
    nc.compile()
    return nc, batches


def kernel(x_re, x_im, R_re, R_im, edge_index, edge_type):
    x_re = np.asarray(x_re, dtype=np.float32)
    x_im = np.asarray(x_im, dtype=np.float32)
    R_re = np.asarray(R_re, dtype=np.float32)
    R_im = np.asarray(R_im, dtype=np.float32)
    src = np.asarray(edge_index[0], dtype=np.int64)
    dst = np.asarray(edge_index[1], dtype=np.int64)
    typ = np.asarray(edge_type, dtype=np.int64)

    n_nodes, d = x_re.shape
    n_rel = R_re.shape[0]
    n_edges = src.shape[0]
    assert d * 2 == D2
    n_chunks = (n_nodes + CHUNK - 1) // CHUNK

    xcat = np.concatenate([x_re, x_im], axis=1)
    xneg = np.concatenate([x_re, -x_im], axis=1)
    rcat = np.concatenate([R_re, -R_im], axis=1)
    # paired relation table: one 1KB gather descriptor serves two edges
    rpair = np.concatenate(
        [np.repeat(rcat, n_rel, axis=0), np.tile(rcat, (n_rel, 1))], axis=1
    ).astype(np.float32)

    # ---- deal edges to cores per (src_chunk, dst_chunk) bin ----
    # Round-robin within each bin equalizes per-core bin counts (spread <= 1),
    # minimizing the shared bin capacities and thus padded descriptors.
    n_bins = n_chunks * n_chunks
    bin_id = (src // CHUNK) * n_chunks + (dst // CHUNK)
    order = np.argsort(bin_id, kind="stable")
    counts = np.bincount(bin_id, minlength=n_bins)
    ends = np.cumsum(counts)
    rank_in_bin = np.empty(n_edges, dtype=np.int64)
    rank_in_bin[order] = np.arange(n_edges) - np.concatenate([[0], ends])[bin_id[order]]
    core_of = rank_in_bin % N_CORES
    pos_in_bin = rank_in_bin // N_CORES

    per_core_max = (counts + N_CORES - 1) // N_CORES
    bin_caps = ((per_core_max + 255) // 256 * 256).astype(np.int64)
    e_pad = int(bin_caps.sum())
    bin_starts = np.concatenate([[0], np.cumsum(bin_caps)])
    stream_pos = bin_starts[bin_id] + pos_in_bin  # per-edge slot in its core's stream

    key = (n_nodes, n_rel, e_pad, tuple(bin_caps.tolist()))
    if key not in _cache:
        _cache[key] = _build(n_nodes, n_rel, e_pad, bin_caps, n_chunks)
    nc, _batches = _cache[key]

    # ---- build per-core padded index streams ----
    def wrap16(a):
        w = a.reshape(-1, 16).T  # [16, len/16]
        return np.tile(w, (8, 1)).copy()

    in_maps = []
    for c in range(N_CORES):
        m = core_of == c
        sp = stream_pos[m]
        # pad slots default to chunk-local row 0 / relation 0 (always valid)
        ls = np.zeros(e_pad, dtype=np.int16)
        lo_ = np.zeros(e_pad, dtype=np.int16)
        lr = np.zeros(e_pad, dtype=np.int16)
        ls[sp] = (src[m] % CHUNK).astype(np.int16)
        lo_[sp] = (dst[m] % CHUNK).astype(np.int16)
        lr[sp] = typ[m].astype(np.int16)
        # pair consecutive stream columns: rc desc j covers stream positions
        # (2c*128+p, (2c+1)*128+p); idx = t_a * n_rel + t_b
        T = lr.reshape(-1, P)
        lrp = (T[0::2].astype(np.int32) * n_rel + T[1::2]).astype(np.int16).reshape(-1)
        in_maps.append(
            {
                "xcat": xcat,
                "xneg": xneg,
                "rpair": rpair,
                "idx_s": wrap16(ls),
                "idx_o": wrap16(lo_),
                "idx_r": wrap16(lrp),
            }
        )

    res = run_bass_kernel_spmd(nc, in_maps, core_ids=list(range(N_CORES)))
    global _last_results
    _last_results = res

    # ---- unpermute: stream position i -> out[i % 128, i // 128] ----
    scores = np.empty(n_edges, dtype=np.float32)
    for c in range(N_CORES):
        grid = res.results[c]["out"]  # [128, e_pad//128]
        stream = grid.T.reshape(-1)  # stream[i] = score of stream position i
        m = core_of == c
        scores[m] = stream[stream_pos[m]]
    return scores



# revision 5
# speedup vs baseline: 2.1863x; 2.1863x over previous
"""ComplEx decoder edge scoring on 8 Trainium2 NeuronCores.

score[e] = sum_d Re( s_e * r_e * o_e )  for complex embeddings
         = sum_d [ r_re*(s_re*o_re - s_im*o_im) - r_im*(s_re*o_im + s_im*o_re) ]

Strategy (pure edge parallelism, hint-compliant):
  - Edges dealt round-robin to the 8 cores within each (src_chunk, dst_chunk)
    bin; node/relation tables replicated per core. 32768-row node chunks keep
    chunk-local row ids inside the int16 index format of the SWDGE dma_gather
    instruction. Bin capacities are shared across cores (rounded up to 256)
    so a single SPMD NEFF serves all 8 cores.
  - Device loop per 4096-edge batch: three dma_gather streams pull per-edge
    rows (s 512B from [x_re|x_im], o 512B from [x_re|-x_im], and relation
    rows 1KB from a 100x100 paired table [rc_t1|rc_t2] so one descriptor
    serves two edges), then DVE elementwise complex product + weighting +
    free-dim reduction produces one f32 score per edge.
  - Host side only slices/sorts indices and un-permutes the scores.

Performance note: throughput is pinned by the SWDGE Q7 descriptor-generation
rate (~8 ns/descriptor aggregate, measured; SDMA engines sit ~75% idle), so
the design minimizes descriptor count: 2.5 descriptors per edge.
"""

import os
import numpy as np
from contextlib import ExitStack

import concourse.tile as tile
from concourse import bacc, mybir
from concourse.bass_utils import run_bass_kernel_spmd

N_CORES = 8
CHUNK = 32768          # node-table chunk rows (int16 index limit)
BATCH = 4096           # edges per compute batch
P = 128                # SBUF partitions
D2 = 128               # concat feature dim (2 * D)
N_QUEUES = int(os.environ.get("KQ", "4"))      # SWDGE queues (desc-gen core pairs)
SINGLE_PACKET = os.environ.get("KSP", "0") == "1"

_cache = {}
_last_results = None


def _build(n_nodes, n_rel, e_pad, bin_caps, n_chunks):
    """Compile the SPMD kernel for the given static layout."""
    f32 = mybir.dt.float32
    i16 = mybir.dt.int16

    # batch boundaries and per-batch gather segments (cut at bin boundaries)
    bin_starts = np.concatenate([[0], np.cumsum(bin_caps)])
    batches = []
    pos = 0
    while pos < e_pad:
        blen = min(BATCH, e_pad - pos)
        # segments: [pos, pos+blen) cut at bin boundaries
        segs = []
        for b in range(len(bin_caps)):
            lo = max(pos, int(bin_starts[b]))
            hi = min(pos + blen, int(bin_starts[b + 1]))
            if lo < hi:
                segs.append((lo, hi - lo, b // n_chunks, b % n_chunks))
        batches.append((pos, blen, segs))
        pos += blen

    nc = bacc.Bacc("TRN2", num_swdge_queues=N_QUEUES)
    qctr = [0]

    def next_q():
        q = qctr[0] % N_QUEUES
        qctr[0] += 1
        return q
    xcat = nc.dram_tensor("xcat", [n_nodes, D2], f32, kind="ExternalInput")
    xneg = nc.dram_tensor("xneg", [n_nodes, D2], f32, kind="ExternalInput")
    rpair = nc.dram_tensor("rpair", [n_rel * n_rel, 2 * D2], f32, kind="ExternalInput")
    idx_s = nc.dram_tensor("idx_s", [P, e_pad // 16], i16, kind="ExternalInput")
    idx_o = nc.dram_tensor("idx_o", [P, e_pad // 16], i16, kind="ExternalInput")
    idx_r = nc.dram_tensor("idx_r", [P, e_pad // 32], i16, kind="ExternalInput")
    out = nc.dram_tensor("out", [P, e_pad // 128], f32, kind="ExternalOutput")

    with ExitStack() as ctx:
        tc = ctx.enter_context(tile.TileContext(nc))
        ipool = ctx.enter_context(tc.tile_pool(name="idx", bufs=3))
        gpool = ctx.enter_context(tc.tile_pool(name="gath", bufs=2))
        tpool = ctx.enter_context(tc.tile_pool(name="tmp", bufs=1))
        spool = ctx.enter_context(tc.tile_pool(name="scores", bufs=1))

        scores = spool.tile([P, e_pad // 128], f32)

        for pos, blen, segs in batches:
            g = blen // 128
            cols = blen // 16
            it_s = ipool.tile([P, cols], i16, tag="it_s")
            it_o = ipool.tile([P, cols], i16, tag="it_o")
            it_r = ipool.tile([P, cols // 2], i16, tag="it_r")
            nc.sync.dma_start(it_s[:], idx_s[:, pos // 16 : pos // 16 + cols])
            nc.sync.dma_start(it_o[:], idx_o[:, pos // 16 : pos // 16 + cols])
            nc.sync.dma_start(it_r[:], idx_r[:, pos // 32 : pos // 32 + cols // 2])

            S = gpool.tile([P, g, D2], f32, tag="S")
            O = gpool.tile([P, g, D2], f32, tag="O")
            RC = gpool.tile([P, g, D2], f32, tag="RC")
            for (L, n, cs, co) in segs:
                g0 = (L - pos) // 128
                c0 = (L - pos) // 16
                sl_s = xcat[cs * CHUNK : min((cs + 1) * CHUNK, n_nodes), :]
                sl_o = xneg[co * CHUNK : min((co + 1) * CHUNK, n_nodes), :]
                nc.gpsimd.dma_gather(
                    S[:, g0 : g0 + n // 128, :], sl_s,
                    it_s[:, c0 : c0 + n // 16], n, n, D2,
                    single_packet=SINGLE_PACKET, queue_num=next_q(),
                )
                nc.gpsimd.dma_gather(
                    O[:, g0 : g0 + n // 128, :], sl_o,
                    it_o[:, c0 : c0 + n // 16], n, n, D2,
                    single_packet=SINGLE_PACKET, queue_num=next_q(),
                )
            rc_pairview = RC[:, :g, :].rearrange(
                "p (h two) d -> p h (two d)", two=2
            )
            nc.gpsimd.dma_gather(
                rc_pairview, rpair[:, :], it_r[:, : cols // 2],
                blen // 2, blen // 2, 2 * D2,
                single_packet=SINGLE_PACKET, queue_num=next_q(),
            )

            # PQ[:, :, 0:128]   = S * O           -> [s_re*o_re | -s_im*o_im]
            # PQ[:, :, 128:192] = S_hi * O_lo     ->  s_im*o_re
            # PQ[:, :, 192:256] = S_lo * O_hi     -> -s_re*o_im
            PQ = tpool.tile([P, g, 256], f32, tag="PQ")
            nc.vector.tensor_tensor(
                out=PQ[:, :, 0:128], in0=S[:, :, :], in1=O[:, :, :],
                op=mybir.AluOpType.mult,
            )
            nc.vector.tensor_tensor(
                out=PQ[:, :, 128:192], in0=S[:, :, 64:128], in1=O[:, :, 0:64],
                op=mybir.AluOpType.mult,
            )
            nc.vector.tensor_tensor(
                out=PQ[:, :, 192:256], in0=S[:, :, 0:64], in1=O[:, :, 64:128],
                op=mybir.AluOpType.mult,
            )
            # A = s_re*o_re - s_im*o_im = PQ[0:64] + PQ[64:128]      (add)
            # B = s_im*o_re + s_re*o_im = PQ[128:192] - PQ[192:256]  (subtract)
            AB = tpool.tile([P, g, D2], f32, tag="AB")
            nc.vector.tensor_tensor(
                out=AB[:, :, 0:64], in0=PQ[:, :, 0:64], in1=PQ[:, :, 64:128],
                op=mybir.AluOpType.add,
            )
            nc.vector.tensor_tensor(
                out=AB[:, :, 64:128], in0=PQ[:, :, 128:192], in1=PQ[:, :, 192:256],
                op=mybir.AluOpType.subtract,
            )
            # W = AB * RC, rc rows = [r_re | -r_im]; overwrite PQ's P-half as scratch
            nc.vector.tensor_tensor(
                out=PQ[:, :, 0:128], in0=AB[:, :, :], in1=RC[:, :g, :],
                op=mybir.AluOpType.mult,
            )
            nc.vector.tensor_reduce(
                out=scores[:, pos // 128 : pos // 128 + g],
                in_=PQ[:, :, 0:128],
                axis=mybir.AxisListType.X,
                op=mybir.AluOpType.add,
            )

        nc.sync.dma_start(out[:], scores[:])
    nc.compile()
    return nc, batches


def kernel(x_re, x_im, R_re, R_im, edge_index, edge_type):
    x_re = np.asarray(x_re, dtype=np.float32)
    x_im = np.asarray(x_im, dtype=np.float32)
    R_re = np.asarray(R_re, dtype=np.float32)
    R_im = np.asarray(R_im, dtype=np.float32)
    src = np.asarray(edge_index[0], dtype=np.int64)
    dst = np.asarray(edge_index[1], dtype=np.int64)
    typ = np.asarray(edge_type, dtype=np.int64)

    n_nodes, d = x_re.shape
    n_rel = R_re.shape[0]
    n_edges = src.shape[0]
    assert d * 2 == D2
    n_chunks = (n_nodes + CHUNK - 1) // CHUNK

    xcat = np.concatenate([x_re, x_im], axis=1)
    xneg = np.concatenate([x_re, -x_im], axis=1)
    rcat = np.concatenate([R_re, -R_im], axis=1)
    # paired relation table: one 1KB gather descriptor serves two edges
    rpair = np.concatenate(
        [np.repeat(rcat, n_rel, axis=0), np.tile(rcat, (n_rel, 1))], axis=1
    ).astype(np.float32)

    # ---- deal edges to cores per (src_chunk, dst_chunk) bin ----
    # Round-robin within each bin equalizes per-core bin counts (spread <= 1),
    # minimizing the shared bin capacities and thus padded descriptors.
    n_bins = n_chunks * n_chunks
    bin_id = (src // CHUNK) * n_chunks + (dst // CHUNK)
    order = np.argsort(bin_id, kind="stable")
    counts = np.bincount(bin_id, minlength=n_bins)
    ends = np.cumsum(counts)
    rank_in_bin = np.empty(n_edges, dtype=np.int64)
    rank_in_bin[order] = np.arange(n_edges) - np.concatenate([[0], ends])[bin_id[order]]
    core_of = rank_in_bin % N_CORES
    pos_in_bin = rank_in_bin // N_CORES

    per_core_max = (counts + N_CORES - 1) // N_CORES
    bin_caps = ((per_core_max + 255) // 256 * 256).astype(np.int64)
    e_pad = int(bin_caps.sum())
    bin_starts = np.concatenate([[0], np.cumsum(bin_caps)])
    stream_pos = bin_starts[bin_id] + pos_in_bin  # per-edge slot in its core's stream

    key = (n_nodes, n_rel, e_pad, tuple(bin_caps.tolist()))
    if key not in _cache:
        _cache[key] = _build(n_nodes, n_rel, e_pad, bin_caps, n_chunks)
    nc, _batches = _cache[key]

    # ---- build per-core padded index streams ----
    def wrap16(a):
        w = a.reshape(-1, 16).T  # [16, len/16]
        return np.tile(w, (8, 1)).copy()

    in_maps = []
    for c in range(N_CORES):
        m = core_of == c
        sp = stream_pos[m]
        # pad slots default to chunk-local row 0 / relation 0 (always valid)
        ls = np.zeros(e_pad, dtype=np.int16)
        lo_ = np.zeros(e_pad, dtype=np.int16)
        lr = np.zeros(e_pad, dtype=np.int16)
        ls[sp] = (src[m] % CHUNK).astype(np.int16)
        lo_[sp] = (dst[m] % CHUNK).astype(np.int16)
        lr[sp] = typ[m].astype(np.int16)
        # pair consecutive stream columns: rc desc j covers stream positions
        # (2c*128+p, (2c+1)*128+p); idx = t_a * n_rel + t_b
        T = lr.reshape(-1, P)
        lrp = (T[0::2].astype(np.int32) * n_rel + T[1::2]).astype(np.int16).reshape(-1)
        in_maps.append(
            {
                "xcat": xcat,
                "xneg": xneg,
                "rpair": rpair,
                "idx_s": wrap16(ls),
                "idx_o": wrap16(lo_),
                "idx_r": wrap16(lrp),
            }
        )

    res = run_bass_kernel_spmd(nc, in_maps, core_ids=list(range(N_CORES)))
    global _last_results
    _last_results = res

    # ---- unpermute: stream position i -> out[i % 128, i // 128] ----
    scores = np.empty(n_edges, dtype=np.float32)
    for c in range(N_CORES):
        grid = res.results[c]["out"]  # [128, e_pad//128]
        stream = grid.T.reshape(-1)  # stream[i] = score of stream position i
        m = core_of == c
        scores[m] = stream[stream_pos[m]]
    return scores



# revision 6
# speedup vs baseline: 2.7147x; 1.2417x over previous
"""ComplEx decoder edge scoring on 8 Trainium2 NeuronCores.

score[e] = sum_d Re( s_e * r_e * o_e )  for complex embeddings
         = sum_d [ r_re*(s_re*o_re - s_im*o_im) - r_im*(s_re*o_im + s_im*o_re) ]

Strategy (pure edge parallelism, hint-compliant):
  - Edges dealt round-robin to the 8 cores within each (src_chunk, dst_chunk)
    bin; node/relation tables replicated per core. 32768-row node chunks keep
    chunk-local row ids inside the int16 index format of the SWDGE dma_gather
    instruction. Bin capacities are shared across cores (rounded up to 256)
    so a single SPMD NEFF serves all 8 cores.
  - Device loop per 8192-edge batch: three dma_gather streams pull per-edge
    rows (s 256B from bf16 [x_re|x_im], o 256B from bf16 [x_re|-x_im], and
    relation rows 512B from a 100x100 paired bf16 table [rc_t1|rc_t2] so one
    descriptor serves two edges), then DVE elementwise complex product +
    weighting + free-dim reduction produces one f32 score per edge.
  - Gathers are spread across all 4 SWDGE queues (greedy balance), so
    descriptor generation runs on all 8 GPSIMD Q7 cores instead of 2.
  - Host side only slices/sorts indices and un-permutes the scores.
"""

import os
import numpy as np
import ml_dtypes
from contextlib import ExitStack

import concourse.tile as tile
from concourse import bacc, mybir
from concourse.bass_utils import run_bass_kernel_spmd

N_CORES = 8
CHUNK = 32768          # node-table chunk rows (int16 index limit)
BATCH = 8192           # edges per compute batch
P = 128                # SBUF partitions
D2 = 128               # concat feature dim (2 * D)
N_QUEUES = int(os.environ.get("KQ", "4"))      # SWDGE queues (desc-gen core pairs)
SINGLE_PACKET = os.environ.get("KSP", "0") == "1"

BF16 = mybir.dt.bfloat16
NP_BF16 = ml_dtypes.bfloat16

_cache = {}
_last_results = None


def _build(n_nodes, n_rel, e_pad, bin_caps, n_chunks):
    """Compile the SPMD kernel for the given static layout."""
    f32 = mybir.dt.float32
    i16 = mybir.dt.int16

    # batch boundaries and per-batch gather segments (cut at bin boundaries)
    bin_starts = np.concatenate([[0], np.cumsum(bin_caps)])
    batches = []
    pos = 0
    while pos < e_pad:
        blen = min(BATCH, e_pad - pos)
        segs = []
        for b in range(len(bin_caps)):
            lo = max(pos, int(bin_starts[b]))
            hi = min(pos + blen, int(bin_starts[b + 1]))
            if lo < hi:
                segs.append((lo, hi - lo, b // n_chunks, b % n_chunks))
        batches.append((pos, blen, segs))
        pos += blen

    nc = bacc.Bacc("TRN2", num_swdge_queues=N_QUEUES)
    qload = [0] * N_QUEUES

    def next_q(ndesc):
        q = qload.index(min(qload))
        qload[q] += ndesc
        return q

    xcat = nc.dram_tensor("xcat", [n_nodes, D2], BF16, kind="ExternalInput")
    xneg = nc.dram_tensor("xneg", [n_nodes, D2], BF16, kind="ExternalInput")
    rpair = nc.dram_tensor("rpair", [n_rel * n_rel, 2 * D2], BF16, kind="ExternalInput")
    idx_s = nc.dram_tensor("idx_s", [P, e_pad // 16], i16, kind="ExternalInput")
    idx_o = nc.dram_tensor("idx_o", [P, e_pad // 16], i16, kind="ExternalInput")
    idx_r = nc.dram_tensor("idx_r", [P, e_pad // 32], i16, kind="ExternalInput")
    out = nc.dram_tensor("out", [P, e_pad // 128], f32, kind="ExternalOutput")

    with ExitStack() as ctx:
        tc = ctx.enter_context(tile.TileContext(nc))
        ipool = ctx.enter_context(tc.tile_pool(name="idx", bufs=3))
        gpool = ctx.enter_context(tc.tile_pool(name="gath", bufs=2))
        tpool = ctx.enter_context(tc.tile_pool(name="tmp", bufs=2))
        spool = ctx.enter_context(tc.tile_pool(name="scores", bufs=1))

        scores = spool.tile([P, e_pad // 128], f32)

        for pos, blen, segs in batches:
            g = blen // 128
            cols = blen // 16
            it_s = ipool.tile([P, cols], i16, tag="it_s")
            it_o = ipool.tile([P, cols], i16, tag="it_o")
            it_r = ipool.tile([P, cols // 2], i16, tag="it_r")
            nc.sync.dma_start(it_s[:], idx_s[:, pos // 16 : pos // 16 + cols])
            nc.sync.dma_start(it_o[:], idx_o[:, pos // 16 : pos // 16 + cols])
            nc.sync.dma_start(it_r[:], idx_r[:, pos // 32 : pos // 32 + cols // 2])

            S = gpool.tile([P, g, D2], BF16, tag="S")
            O = gpool.tile([P, g, D2], BF16, tag="O")
            RC = gpool.tile([P, g, D2], BF16, tag="RC")
            for (L, n, cs, co) in segs:
                g0 = (L - pos) // 128
                c0 = (L - pos) // 16
                sl_s = xcat[cs * CHUNK : min((cs + 1) * CHUNK, n_nodes), :]
                sl_o = xneg[co * CHUNK : min((co + 1) * CHUNK, n_nodes), :]
                nc.gpsimd.dma_gather(
                    S[:, g0 : g0 + n // 128, :], sl_s,
                    it_s[:, c0 : c0 + n // 16], n, n, D2,
                    single_packet=SINGLE_PACKET, queue_num=next_q(n),
                )
                nc.gpsimd.dma_gather(
                    O[:, g0 : g0 + n // 128, :], sl_o,
                    it_o[:, c0 : c0 + n // 16], n, n, D2,
                    single_packet=SINGLE_PACKET, queue_num=next_q(n),
                )
            rc_pairview = RC[:, :g, :].rearrange(
                "p (h two) d -> p h (two d)", two=2
            )
            nc.gpsimd.dma_gather(
                rc_pairview, rpair[:, :], it_r[:, : cols // 2],
                blen // 2, blen // 2, 2 * D2,
                single_packet=SINGLE_PACKET, queue_num=next_q(blen // 2),
            )

            # PQ[:, :, 0:128]   = S * O           -> [s_re*o_re | -s_im*o_im]
            # PQ[:, :, 128:192] = S_hi * O_lo     ->  s_im*o_re
            # PQ[:, :, 192:256] = S_lo * O_hi     -> -s_re*o_im
            PQ = tpool.tile([P, g, 256], BF16, tag="PQ")
            nc.vector.tensor_tensor(
                out=PQ[:, :, 0:128], in0=S[:, :, :], in1=O[:, :, :],
                op=mybir.AluOpType.mult,
            )
            nc.vector.tensor_tensor(
                out=PQ[:, :, 128:192], in0=S[:, :, 64:128], in1=O[:, :, 0:64],
                op=mybir.AluOpType.mult,
            )
            nc.vector.tensor_tensor(
                out=PQ[:, :, 192:256], in0=S[:, :, 0:64], in1=O[:, :, 64:128],
                op=mybir.AluOpType.mult,
            )
            # A = s_re*o_re - s_im*o_im = PQ[0:64] + PQ[64:128]      (add)
            # B = s_im*o_re + s_re*o_im = PQ[128:192] - PQ[192:256]  (subtract)
            AB = tpool.tile([P, g, D2], BF16, tag="AB")
            nc.vector.tensor_tensor(
                out=AB[:, :, 0:64], in0=PQ[:, :, 0:64], in1=PQ[:, :, 64:128],
                op=mybir.AluOpType.add,
            )
            nc.vector.tensor_tensor(
                out=AB[:, :, 64:128], in0=PQ[:, :, 128:192], in1=PQ[:, :, 192:256],
                op=mybir.AluOpType.subtract,
            )
            # W = AB * RC, rc rows = [r_re | -r_im]; overwrite PQ's P-half as scratch
            nc.vector.tensor_tensor(
                out=PQ[:, :, 0:128], in0=AB[:, :, :], in1=RC[:, :g, :],
                op=mybir.AluOpType.mult,
            )
            nc.vector.tensor_reduce(
                out=scores[:, pos // 128 : pos // 128 + g],
                in_=PQ[:, :, 0:128],
                axis=mybir.AxisListType.X,
                op=mybir.AluOpType.add,
            )

        nc.sync.dma_start(out[:], scores[:])
    nc.compile()
    return nc, batches


def kernel(x_re, x_im, R_re, R_im, edge_index, edge_type):
    x_re = np.asarray(x_re, dtype=np.float32)
    x_im = np.asarray(x_im, dtype=np.float32)
    R_re = np.asarray(R_re, dtype=np.float32)
    R_im = np.asarray(R_im, dtype=np.float32)
    src = np.asarray(edge_index[0], dtype=np.int64)
    dst = np.asarray(edge_index[1], dtype=np.int64)
    typ = np.asarray(edge_type, dtype=np.int64)

    n_nodes, d = x_re.shape
    n_rel = R_re.shape[0]
    n_edges = src.shape[0]
    assert d * 2 == D2
    n_chunks = (n_nodes + CHUNK - 1) // CHUNK

    xcat = np.concatenate([x_re, x_im], axis=1).astype(NP_BF16)
    xneg = np.concatenate([x_re, -x_im], axis=1).astype(NP_BF16)
    rcat = np.concatenate([R_re, -R_im], axis=1)
    # paired relation table: one 512B gather descriptor serves two edges
    rpair = np.concatenate(
        [np.repeat(rcat, n_rel, axis=0), np.tile(rcat, (n_rel, 1))], axis=1
    ).astype(NP_BF16)

    # ---- deal edges to cores per (src_chunk, dst_chunk) bin ----
    # Round-robin within each bin equalizes per-core bin counts (spread <= 1),
    # minimizing the shared bin capacities and thus padded descriptors.
    n_bins = n_chunks * n_chunks
    bin_id = (src // CHUNK) * n_chunks + (dst // CHUNK)
    order = np.argsort(bin_id, kind="stable")
    counts = np.bincount(bin_id, minlength=n_bins)
    ends = np.cumsum(counts)
    rank_in_bin = np.empty(n_edges, dtype=np.int64)
    rank_in_bin[order] = np.arange(n_edges) - np.concatenate([[0], ends])[bin_id[order]]
    core_of = rank_in_bin % N_CORES
    pos_in_bin = rank_in_bin // N_CORES

    per_core_max = (counts + N_CORES - 1) // N_CORES
    bin_caps = ((per_core_max + 255) // 256 * 256).astype(np.int64)
    e_pad = int(bin_caps.sum())
    bin_starts = np.concatenate([[0], np.cumsum(bin_caps)])
    stream_pos = bin_starts[bin_id] + pos_in_bin  # per-edge slot in its core's stream

    key = (n_nodes, n_rel, e_pad, tuple(bin_caps.tolist()))
    if key not in _cache:
        _cache[key] = _build(n_nodes, n_rel, e_pad, bin_caps, n_chunks)
    nc, _batches = _cache[key]

    # ---- build per-core padded index streams ----
    def wrap16(a):
        w = a.reshape(-1, 16).T  # [16, len/16]
        return np.tile(w, (8, 1)).copy()

    in_maps = []
    for c in range(N_CORES):
        m = core_of == c
        sp = stream_pos[m]
        # pad slots default to chunk-local row 0 / relation 0 (always valid)
        ls = np.zeros(e_pad, dtype=np.int16)
        lo_ = np.zeros(e_pad, dtype=np.int16)
        lr = np.zeros(e_pad, dtype=np.int16)
        ls[sp] = (src[m] % CHUNK).astype(np.int16)
        lo_[sp] = (dst[m] % CHUNK).astype(np.int16)
        lr[sp] = typ[m].astype(np.int16)
        # pair consecutive stream columns: rc desc j covers stream positions
        # (2c*128+p, (2c+1)*128+p); idx = t_a * n_rel + t_b
        T = lr.reshape(-1, P)
        lrp = (T[0::2].astype(np.int32) * n_rel + T[1::2]).astype(np.int16).reshape(-1)
        in_maps.append(
            {
                "xcat": xcat,
                "xneg": xneg,
                "rpair": rpair,
                "idx_s": wrap16(ls),
                "idx_o": wrap16(lo_),
                "idx_r": wrap16(lrp),
            }
        )

    res = run_bass_kernel_spmd(nc, in_maps, core_ids=list(range(N_CORES)))
    global _last_results
    _last_results = res

    # ---- unpermute: stream position i -> out[i % 128, i // 128] ----
    scores = np.empty(n_edges, dtype=np.float32)
    for c in range(N_CORES):
        grid = res.results[c]["out"]  # [128, e_pad//128]
        stream = grid.T.reshape(-1)  # stream[i] = score of stream position i
        m = core_of == c
        scores[m] = stream[stream_pos[m]]
    return scores


# revision 11
# speedup vs baseline: 2.7997x; 1.0313x over previous
"""ComplEx decoder edge scoring on 8 Trainium2 NeuronCores.

score[e] = sum_d Re( s_e * r_e * o_e )  for complex embeddings
         = sum_d [ r_re*(s_re*o_re - s_im*o_im) - r_im*(s_re*o_im + s_im*o_re) ]

Strategy (pure edge parallelism, hint-compliant):
  - Edges dealt round-robin to the 8 cores within each (src_chunk, dst_chunk)
    bin; node/relation tables replicated per core. 32768-row node chunks keep
    chunk-local row ids inside the int16 index format of the SWDGE dma_gather
    instruction. Bin capacities are shared across cores (rounded up to 256)
    so a single SPMD NEFF serves all 8 cores.
  - Device loop per 8192-edge batch: three dma_gather streams pull per-edge
    rows (s 256B from bf16 [x_re|x_im], o 256B from bf16 [x_re|-x_im], and
    relation rows 512B from a 100x100 paired bf16 table [rc_t1|rc_t2] so one
    descriptor serves two edges), then DVE elementwise complex product +
    weighting + free-dim reduction produces one f32 score per edge.
  - Gathers are spread across all 4 SWDGE queues (greedy balance), so
    descriptor generation runs on all 8 GPSIMD Q7 cores instead of 2.
  - Host side only slices/sorts indices and un-permutes the scores.
"""

import os
import numpy as np
import ml_dtypes
from contextlib import ExitStack

import concourse.tile as tile
from concourse import bacc, mybir
from concourse.bass_utils import run_bass_kernel_spmd

N_CORES = 8
CHUNK = 32768          # node-table chunk rows (int16 index limit)
BATCH = 8192           # edges per compute batch
P = 128                # SBUF partitions
D2 = 128               # concat feature dim (2 * D)
N_QUEUES = int(os.environ.get("KQ", "4"))      # SWDGE queues (desc-gen core pairs)
SINGLE_PACKET = os.environ.get("KSP", "0") == "1"

BF16 = mybir.dt.bfloat16
NP_BF16 = ml_dtypes.bfloat16

_cache = {}
_last_results = None


def _build(n_nodes, n_rel, e_pad, bin_caps, n_chunks):
    """Compile the SPMD kernel for the given static layout."""
    f32 = mybir.dt.float32
    i16 = mybir.dt.int16

    # batch boundaries and per-batch gather segments (cut at bin boundaries)
    bin_starts = np.concatenate([[0], np.cumsum(bin_caps)])
    batches = []
    pos = 0
    while pos < e_pad:
        blen = min(BATCH, e_pad - pos)
        segs = []
        for b in range(len(bin_caps)):
            lo = max(pos, int(bin_starts[b]))
            hi = min(pos + blen, int(bin_starts[b + 1]))
            if lo < hi:
                segs.append((lo, hi - lo, b // n_chunks, b % n_chunks))
        batches.append((pos, blen, segs))
        pos += blen

    nc = bacc.Bacc("TRN2", num_swdge_queues=N_QUEUES)
    qload = [0] * N_QUEUES

    def next_q(ndesc):
        q = qload.index(min(qload))
        qload[q] += ndesc
        return q

    xcat = nc.dram_tensor("xcat", [n_nodes, D2], BF16, kind="ExternalInput")
    xneg = nc.dram_tensor("xneg", [n_nodes, D2], BF16, kind="ExternalInput")
    # per-edge relation rows [r_re | -r_im] pre-ordered on host into stream
    # order: rc_seq[p, h, :] is the row for stream position h*128+p. Loaded
    # with one sequential HWDGE DMA per batch (no gather descriptors).
    rc_seq = nc.dram_tensor(
        "rc_seq", [P, e_pad // 128, D2], BF16, kind="ExternalInput"
    )
    idx_s = nc.dram_tensor("idx_s", [P, e_pad // 16], i16, kind="ExternalInput")
    idx_o = nc.dram_tensor("idx_o", [P, e_pad // 16], i16, kind="ExternalInput")
    out = nc.dram_tensor("out", [P, e_pad // 128], f32, kind="ExternalOutput")

    with ExitStack() as ctx:
        tc = ctx.enter_context(tile.TileContext(nc))
        ipool = ctx.enter_context(tc.tile_pool(name="idx", bufs=3))
        gpool = ctx.enter_context(tc.tile_pool(name="gath", bufs=2))
        tpool = ctx.enter_context(tc.tile_pool(name="tmp", bufs=2))
        spool = ctx.enter_context(tc.tile_pool(name="scores", bufs=1))

        scores = spool.tile([P, e_pad // 128], f32)

        for pos, blen, segs in batches:
            g = blen // 128
            cols = blen // 16
            it_s = ipool.tile([P, cols], i16, tag="it_s")
            it_o = ipool.tile([P, cols], i16, tag="it_o")
            nc.sync.dma_start(it_s[:], idx_s[:, pos // 16 : pos // 16 + cols])
            nc.sync.dma_start(it_o[:], idx_o[:, pos // 16 : pos // 16 + cols])

            S = gpool.tile([P, g, D2], BF16, tag="S")
            O = gpool.tile([P, g, D2], BF16, tag="O")
            RC = gpool.tile([P, g, D2], BF16, tag="RC")
            for (L, n, cs, co) in segs:
                g0 = (L - pos) // 128
                c0 = (L - pos) // 16
                sl_s = xcat[cs * CHUNK : min((cs + 1) * CHUNK, n_nodes), :]
                sl_o = xneg[co * CHUNK : min((co + 1) * CHUNK, n_nodes), :]
                nc.gpsimd.dma_gather(
                    S[:, g0 : g0 + n // 128, :], sl_s,
                    it_s[:, c0 : c0 + n // 16], n, n, D2,
                    single_packet=SINGLE_PACKET, queue_num=next_q(n),
                )
                nc.gpsimd.dma_gather(
                    O[:, g0 : g0 + n // 128, :], sl_o,
                    it_o[:, c0 : c0 + n // 16], n, n, D2,
                    single_packet=SINGLE_PACKET, queue_num=next_q(n),
                )
            nc.sync.dma_start(
                RC[:, :g, :], rc_seq[:, pos // 128 : pos // 128 + g, :]
            )

            # PQ[:, :, 0:128]   = S * O           -> [s_re*o_re | -s_im*o_im]
            # PQ[:, :, 128:192] = S_hi * O_lo     ->  s_im*o_re
            # PQ[:, :, 192:256] = S_lo * O_hi     -> -s_re*o_im
            PQ = tpool.tile([P, g, 256], BF16, tag="PQ")
            nc.vector.tensor_tensor(
                out=PQ[:, :, 0:128], in0=S[:, :, :], in1=O[:, :, :],
                op=mybir.AluOpType.mult,
            )
            nc.vector.tensor_tensor(
                out=PQ[:, :, 128:192], in0=S[:, :, 64:128], in1=O[:, :, 0:64],
                op=mybir.AluOpType.mult,
            )
            nc.vector.tensor_tensor(
                out=PQ[:, :, 192:256], in0=S[:, :, 0:64], in1=O[:, :, 64:128],
                op=mybir.AluOpType.mult,
            )
            # A = s_re*o_re - s_im*o_im = PQ[0:64] + PQ[64:128]      (add)
            # B = s_im*o_re + s_re*o_im = PQ[128:192] - PQ[192:256]  (subtract)
            AB = tpool.tile([P, g, D2], BF16, tag="AB")
            nc.vector.tensor_tensor(
                out=AB[:, :, 0:64], in0=PQ[:, :, 0:64], in1=PQ[:, :, 64:128],
                op=mybir.AluOpType.add,
            )
            nc.vector.tensor_tensor(
                out=AB[:, :, 64:128], in0=PQ[:, :, 128:192], in1=PQ[:, :, 192:256],
                op=mybir.AluOpType.subtract,
            )
            # W = AB * RC, rc rows = [r_re | -r_im]; overwrite PQ's P-half as scratch
            nc.vector.tensor_tensor(
                out=PQ[:, :, 0:128], in0=AB[:, :, :], in1=RC[:, :g, :],
                op=mybir.AluOpType.mult,
            )
            nc.vector.tensor_reduce(
                out=scores[:, pos // 128 : pos // 128 + g],
                in_=PQ[:, :, 0:128],
                axis=mybir.AxisListType.X,
                op=mybir.AluOpType.add,
            )

        nc.sync.dma_start(out[:], scores[:])
    nc.compile()
    return nc, batches


def kernel(x_re, x_im, R_re, R_im, edge_index, edge_type):
    x_re = np.asarray(x_re, dtype=np.float32)
    x_im = np.asarray(x_im, dtype=np.float32)
    R_re = np.asarray(R_re, dtype=np.float32)
    R_im = np.asarray(R_im, dtype=np.float32)
    src = np.asarray(edge_index[0], dtype=np.int64)
    dst = np.asarray(edge_index[1], dtype=np.int64)
    typ = np.asarray(edge_type, dtype=np.int64)

    n_nodes, d = x_re.shape
    n_rel = R_re.shape[0]
    n_edges = src.shape[0]
    assert d * 2 == D2
    n_chunks = (n_nodes + CHUNK - 1) // CHUNK

    xcat = np.concatenate([x_re, x_im], axis=1).astype(NP_BF16)
    xneg = np.concatenate([x_re, -x_im], axis=1).astype(NP_BF16)
    rcat16 = np.concatenate([R_re, -R_im], axis=1).astype(NP_BF16)

    # ---- deal edges to cores per (src_chunk, dst_chunk) bin ----
    # Round-robin within each bin equalizes per-core bin counts (spread <= 1),
    # minimizing the shared bin capacities and thus padded descriptors.
    n_bins = n_chunks * n_chunks
    bin_id = (src // CHUNK) * n_chunks + (dst // CHUNK)
    order = np.argsort(bin_id, kind="stable")
    counts = np.bincount(bin_id, minlength=n_bins)
    ends = np.cumsum(counts)
    rank_in_bin = np.empty(n_edges, dtype=np.int64)
    rank_in_bin[order] = np.arange(n_edges) - np.concatenate([[0], ends])[bin_id[order]]
    core_of = rank_in_bin % N_CORES
    pos_in_bin = rank_in_bin // N_CORES

    per_core_max = (counts + N_CORES - 1) // N_CORES
    bin_caps = ((per_core_max + 255) // 256 * 256).astype(np.int64)
    e_pad = int(bin_caps.sum())
    bin_starts = np.concatenate([[0], np.cumsum(bin_caps)])
    stream_pos = bin_starts[bin_id] + pos_in_bin  # per-edge slot in its core's stream

    key = (n_nodes, n_rel, e_pad, tuple(bin_caps.tolist()))
    if key not in _cache:
        _cache[key] = _build(n_nodes, n_rel, e_pad, bin_caps, n_chunks)
    nc, _batches = _cache[key]

    # ---- build per-core padded index streams ----
    def wrap16(a):
        w = a.reshape(-1, 16).T  # [16, len/16]
        return np.tile(w, (8, 1)).copy()

    in_maps = []
    for c in range(N_CORES):
        m = core_of == c
        sp = stream_pos[m]
        # pad slots default to chunk-local row 0 / relation 0 (always valid)
        ls = np.zeros(e_pad, dtype=np.int16)
        lo_ = np.zeros(e_pad, dtype=np.int16)
        lr = np.zeros(e_pad, dtype=np.int16)
        ls[sp] = (src[m] % CHUNK).astype(np.int16)
        lo_[sp] = (dst[m] % CHUNK).astype(np.int16)
        lr[sp] = typ[m].astype(np.int16)
        # relation rows in stream order: [128, e_pad//128, D2]
        rc_core = rcat16[lr].reshape(e_pad // 128, P, D2).transpose(1, 0, 2).copy()
        in_maps.append(
            {
                "xcat": xcat,
                "xneg": xneg,
                "rc_seq": rc_core,
                "idx_s": wrap16(ls),
                "idx_o": wrap16(lo_),
            }
        )

    res = run_bass_kernel_spmd(nc, in_maps, core_ids=list(range(N_CORES)))
    global _last_results
    _last_results = res

    # ---- unpermute: stream position i -> out[i % 128, i // 128] ----
    scores = np.empty(n_edges, dtype=np.float32)
    for c in range(N_CORES):
        grid = res.results[c]["out"]  # [128, e_pad//128]
        stream = grid.T.reshape(-1)  # stream[i] = score of stream position i
        m = core_of == c
        scores[m] = stream[stream_pos[m]]
    return scores


# revision 13
# speedup vs baseline: 3.0786x; 1.0997x over previous
"""ComplEx decoder edge scoring on 8 Trainium2 NeuronCores.

score[e] = sum_d Re( s_e * r_e * o_e )  for complex embeddings
         = sum_d [ r_re*(s_re*o_re - s_im*o_im) - r_im*(s_re*o_im + s_im*o_re) ]

Strategy (pure edge parallelism, hint-compliant):
  - Edges dealt round-robin to the 8 cores within each (src_chunk, dst_chunk)
    bin; node/relation tables replicated per core. 32768-row node chunks keep
    chunk-local row ids inside the int16 index format of the SWDGE dma_gather
    instruction. Bin capacities are shared across cores (rounded up to 256)
    so a single SPMD NEFF serves all 8 cores.
  - Device loop per 8192-edge batch: three dma_gather streams pull per-edge
    rows (s 256B from bf16 [x_re|x_im], o 256B from bf16 [x_re|-x_im], and
    relation rows 512B from a 100x100 paired bf16 table [rc_t1|rc_t2] so one
    descriptor serves two edges), then DVE elementwise complex product +
    weighting + free-dim reduction produces one f32 score per edge.
  - Gathers are spread across all 4 SWDGE queues (greedy balance), so
    descriptor generation runs on all 8 GPSIMD Q7 cores instead of 2.
  - Host side only slices/sorts indices and un-permutes the scores.
"""

import os
import numpy as np
import ml_dtypes
from contextlib import ExitStack

import concourse.tile as tile
from concourse import bacc, mybir
from concourse.bass_utils import run_bass_kernel_spmd

N_CORES = 8
CHUNK = 32768          # node-table chunk rows (int16 index limit)
BATCH = 8192           # edges per compute batch
P = 128                # SBUF partitions
D2 = 128               # concat feature dim (2 * D)
N_QUEUES = int(os.environ.get("KQ", "4"))      # SWDGE queues (desc-gen core pairs)
SINGLE_PACKET = os.environ.get("KSP", "0") == "1"

BF16 = mybir.dt.bfloat16
NP_BF16 = ml_dtypes.bfloat16

_cache = {}
_last_results = None


def _build(n_nodes, n_rel, e_pad, bin_caps, n_chunks):
    """Compile the SPMD kernel for the given static layout."""
    f32 = mybir.dt.float32
    i16 = mybir.dt.int16

    # batch boundaries and per-batch gather segments (cut at bin boundaries)
    bin_starts = np.concatenate([[0], np.cumsum(bin_caps)])
    batches = []
    pos = 0
    while pos < e_pad:
        blen = min(BATCH, e_pad - pos)
        segs = []
        for b in range(len(bin_caps)):
            lo = max(pos, int(bin_starts[b]))
            hi = min(pos + blen, int(bin_starts[b + 1]))
            if lo < hi:
                segs.append((lo, hi - lo, b // n_chunks, b % n_chunks))
        batches.append((pos, blen, segs))
        pos += blen

    nc = bacc.Bacc(
        "TRN2",
        num_swdge_queues=N_QUEUES,
        dynamic_dma_scratch_size=int(os.environ.get("KSCRATCH", "32768")),
    )
    qload = [0] * N_QUEUES

    def next_q(ndesc):
        q = qload.index(min(qload))
        qload[q] += ndesc
        return q

    xcat = nc.dram_tensor("xcat", [n_nodes, D2], BF16, kind="ExternalInput")
    xneg = nc.dram_tensor("xneg", [n_nodes, D2], BF16, kind="ExternalInput")
    # per-edge relation rows [r_re | -r_im] pre-ordered on host into stream
    # order: rc_seq[p, h, :] is the row for stream position h*128+p. Loaded
    # with one sequential HWDGE DMA per batch (no gather descriptors).
    rc_seq = nc.dram_tensor(
        "rc_seq", [P, e_pad // 128, D2], BF16, kind="ExternalInput"
    )
    idx_s = nc.dram_tensor("idx_s", [P, e_pad // 16], i16, kind="ExternalInput")
    idx_o = nc.dram_tensor("idx_o", [P, e_pad // 16], i16, kind="ExternalInput")
    out = nc.dram_tensor("out", [P, e_pad // 128], f32, kind="ExternalOutput")

    with ExitStack() as ctx:
        tc = ctx.enter_context(tile.TileContext(nc))
        ipool = ctx.enter_context(tc.tile_pool(name="idx", bufs=3))
        gpool = ctx.enter_context(tc.tile_pool(name="gath", bufs=2))
        tpool = ctx.enter_context(tc.tile_pool(name="tmp", bufs=1))
        spool = ctx.enter_context(tc.tile_pool(name="scores", bufs=1))

        scores = spool.tile([P, e_pad // 128], f32)

        for pos, blen, segs in batches:
            g = blen // 128
            cols = blen // 16
            it_s = ipool.tile([P, cols], i16, tag="it_s")
            it_o = ipool.tile([P, cols], i16, tag="it_o")
            nc.sync.dma_start(it_s[:], idx_s[:, pos // 16 : pos // 16 + cols])
            nc.sync.dma_start(it_o[:], idx_o[:, pos // 16 : pos // 16 + cols])

            S = gpool.tile([P, g, D2], BF16, tag="S")
            O = gpool.tile([P, g, D2], BF16, tag="O")
            RC = gpool.tile([P, g, D2], BF16, tag="RC")
            for (L, n, cs, co) in segs:
                g0 = (L - pos) // 128
                c0 = (L - pos) // 16
                sl_s = xcat[cs * CHUNK : min((cs + 1) * CHUNK, n_nodes), :]
                sl_o = xneg[co * CHUNK : min((co + 1) * CHUNK, n_nodes), :]
                nc.gpsimd.dma_gather(
                    S[:, g0 : g0 + n // 128, :], sl_s,
                    it_s[:, c0 : c0 + n // 16], n, n, D2,
                    single_packet=SINGLE_PACKET, queue_num=next_q(n),
                )
                nc.gpsimd.dma_gather(
                    O[:, g0 : g0 + n // 128, :], sl_o,
                    it_o[:, c0 : c0 + n // 16], n, n, D2,
                    single_packet=SINGLE_PACKET, queue_num=next_q(n),
                )
            nc.sync.dma_start(
                RC[:, :g, :], rc_seq[:, pos // 128 : pos // 128 + g, :]
            )

            # PQ[:, :, 0:128]   = S * O           -> [s_re*o_re | -s_im*o_im]
            # PQ[:, :, 128:192] = S_hi * O_lo     ->  s_im*o_re
            # PQ[:, :, 192:256] = S_lo * O_hi     -> -s_re*o_im
            PQ = tpool.tile([P, g, 256], BF16, tag="PQ")
            nc.vector.tensor_tensor(
                out=PQ[:, :, 0:128], in0=S[:, :, :], in1=O[:, :, :],
                op=mybir.AluOpType.mult,
            )
            nc.vector.tensor_tensor(
                out=PQ[:, :, 128:192], in0=S[:, :, 64:128], in1=O[:, :, 0:64],
                op=mybir.AluOpType.mult,
            )
            nc.vector.tensor_tensor(
                out=PQ[:, :, 192:256], in0=S[:, :, 0:64], in1=O[:, :, 64:128],
                op=mybir.AluOpType.mult,
            )
            # A = s_re*o_re - s_im*o_im = PQ[0:64] + PQ[64:128]      (add)
            # B = s_im*o_re + s_re*o_im = PQ[128:192] - PQ[192:256]  (subtract)
            AB = tpool.tile([P, g, D2], BF16, tag="AB")
            nc.vector.tensor_tensor(
                out=AB[:, :, 0:64], in0=PQ[:, :, 0:64], in1=PQ[:, :, 64:128],
                op=mybir.AluOpType.add,
            )
            nc.vector.tensor_tensor(
                out=AB[:, :, 64:128], in0=PQ[:, :, 128:192], in1=PQ[:, :, 192:256],
                op=mybir.AluOpType.subtract,
            )
            # W = AB * RC, rc rows = [r_re | -r_im]; overwrite PQ's P-half as scratch
            nc.vector.tensor_tensor(
                out=PQ[:, :, 0:128], in0=AB[:, :, :], in1=RC[:, :g, :],
                op=mybir.AluOpType.mult,
            )
            nc.vector.tensor_reduce(
                out=scores[:, pos // 128 : pos // 128 + g],
                in_=PQ[:, :, 0:128],
                axis=mybir.AxisListType.X,
                op=mybir.AluOpType.add,
            )

        nc.sync.dma_start(out[:], scores[:])
    nc.compile()
    return nc, batches


def kernel(x_re, x_im, R_re, R_im, edge_index, edge_type):
    x_re = np.asarray(x_re, dtype=np.float32)
    x_im = np.asarray(x_im, dtype=np.float32)
    R_re = np.asarray(R_re, dtype=np.float32)
    R_im = np.asarray(R_im, dtype=np.float32)
    src = np.asarray(edge_index[0], dtype=np.int64)
    dst = np.asarray(edge_index[1], dtype=np.int64)
    typ = np.asarray(edge_type, dtype=np.int64)

    n_nodes, d = x_re.shape
    n_rel = R_re.shape[0]
    n_edges = src.shape[0]
    assert d * 2 == D2
    n_chunks = (n_nodes + CHUNK - 1) // CHUNK

    xcat = np.concatenate([x_re, x_im], axis=1).astype(NP_BF16)
    xneg = np.concatenate([x_re, -x_im], axis=1).astype(NP_BF16)
    rcat16 = np.concatenate([R_re, -R_im], axis=1).astype(NP_BF16)

    # ---- deal edges to cores per (src_chunk, dst_chunk) bin ----
    # Round-robin within each bin equalizes per-core bin counts (spread <= 1),
    # minimizing the shared bin capacities and thus padded descriptors.
    n_bins = n_chunks * n_chunks
    bin_id = (src // CHUNK) * n_chunks + (dst // CHUNK)
    order = np.argsort(bin_id, kind="stable")
    counts = np.bincount(bin_id, minlength=n_bins)
    ends = np.cumsum(counts)
    rank_in_bin = np.empty(n_edges, dtype=np.int64)
    rank_in_bin[order] = np.arange(n_edges) - np.concatenate([[0], ends])[bin_id[order]]
    core_of = rank_in_bin % N_CORES
    pos_in_bin = rank_in_bin // N_CORES

    per_core_max = (counts + N_CORES - 1) // N_CORES
    bin_caps = ((per_core_max + 255) // 256 * 256).astype(np.int64)
    e_pad = int(bin_caps.sum())
    bin_starts = np.concatenate([[0], np.cumsum(bin_caps)])
    stream_pos = bin_starts[bin_id] + pos_in_bin  # per-edge slot in its core's stream

    key = (n_nodes, n_rel, e_pad, tuple(bin_caps.tolist()))
    if key not in _cache:
        _cache[key] = _build(n_nodes, n_rel, e_pad, bin_caps, n_chunks)
    nc, _batches = _cache[key]

    # ---- build per-core padded index streams ----
    def wrap16(a):
        w = a.reshape(-1, 16).T  # [16, len/16]
        return np.tile(w, (8, 1)).copy()

    in_maps = []
    for c in range(N_CORES):
        m = core_of == c
        sp = stream_pos[m]
        # pad slots default to chunk-local row 0 / relation 0 (always valid)
        ls = np.zeros(e_pad, dtype=np.int16)
        lo_ = np.zeros(e_pad, dtype=np.int16)
        lr = np.zeros(e_pad, dtype=np.int16)
        ls[sp] = (src[m] % CHUNK).astype(np.int16)
        lo_[sp] = (dst[m] % CHUNK).astype(np.int16)
        lr[sp] = typ[m].astype(np.int16)
        # relation rows in stream order: [128, e_pad//128, D2]
        rc_core = rcat16[lr].reshape(e_pad // 128, P, D2).transpose(1, 0, 2).copy()
        in_maps.append(
            {
                "xcat": xcat,
                "xneg": xneg,
                "rc_seq": rc_core,
                "idx_s": wrap16(ls),
                "idx_o": wrap16(lo_),
            }
        )

    res = run_bass_kernel_spmd(nc, in_maps, core_ids=list(range(N_CORES)))
    global _last_results
    _last_results = res

    # ---- unpermute: stream position i -> out[i % 128, i // 128] ----
    scores = np.empty(n_edges, dtype=np.float32)
    for c in range(N_CORES):
        grid = res.results[c]["out"]  # [128, e_pad//128]
        stream = grid.T.reshape(-1)  # stream[i] = score of stream position i
        m = core_of == c
        scores[m] = stream[stream_pos[m]]
    return scores


# revision 14
# speedup vs baseline: 3.6263x; 1.1779x over previous
"""ComplEx decoder edge scoring on 8 Trainium2 NeuronCores.

score[e] = sum_d Re( s_e * r_e * o_e )  for complex embeddings
         = sum_d [ r_re*(s_re*o_re - s_im*o_im) - r_im*(s_re*o_im + s_im*o_re) ]

Strategy (pure edge parallelism, hint-compliant):
  - Edges dealt round-robin to the 8 cores within each (src_chunk, dst_chunk)
    bin; node/relation tables replicated per core. 32768-row node chunks keep
    chunk-local row ids inside the int16 index format of the SWDGE dma_gather
    instruction. Bin capacities are shared across cores (rounded up to 256)
    so a single SPMD NEFF serves all 8 cores.
  - Device loop per 8192-edge batch: three dma_gather streams pull per-edge
    rows (s 256B from bf16 [x_re|x_im], o 256B from bf16 [x_re|-x_im], and
    relation rows 512B from a 100x100 paired bf16 table [rc_t1|rc_t2] so one
    descriptor serves two edges), then DVE elementwise complex product +
    weighting + free-dim reduction produces one f32 score per edge.
  - Gathers are spread across all 4 SWDGE queues (greedy balance), so
    descriptor generation runs on all 8 GPSIMD Q7 cores instead of 2.
  - Host side only slices/sorts indices and un-permutes the scores.
"""

import os
import numpy as np
import ml_dtypes
from contextlib import ExitStack

import concourse.tile as tile
from concourse import bacc, mybir
from concourse.bass_utils import run_bass_kernel_spmd

N_CORES = 8
CHUNK = 32768          # node-table chunk rows (int16 index limit)
BATCH = 8192           # edges per compute batch
P = 128                # SBUF partitions
D2 = 128               # concat feature dim (2 * D)
N_QUEUES = int(os.environ.get("KQ", "4"))      # SWDGE queues (desc-gen core pairs)
SINGLE_PACKET = os.environ.get("KSP", "0") == "1"

BF16 = mybir.dt.bfloat16
NP_BF16 = ml_dtypes.bfloat16

_cache = {}
_last_results = None


def _build(n_nodes, n_rel, e_pad, bin_caps, n_chunks):
    """Compile the SPMD kernel for the given static layout."""
    f32 = mybir.dt.float32
    i16 = mybir.dt.int16

    # batch boundaries and per-batch gather segments (cut at bin boundaries)
    bin_starts = np.concatenate([[0], np.cumsum(bin_caps)])
    batches = []
    pos = 0
    while pos < e_pad:
        blen = min(BATCH, e_pad - pos)
        segs = []
        for b in range(len(bin_caps)):
            lo = max(pos, int(bin_starts[b]))
            hi = min(pos + blen, int(bin_starts[b + 1]))
            if lo < hi:
                segs.append((lo, hi - lo, b // n_chunks, b % n_chunks))
        batches.append((pos, blen, segs))
        pos += blen

    nc = bacc.Bacc(
        "TRN2",
        num_swdge_queues=N_QUEUES,
        dynamic_dma_scratch_size=int(os.environ.get("KSCRATCH", "32768")),
    )
    qload = [0] * N_QUEUES

    def next_q(ndesc):
        q = qload.index(min(qload))
        qload[q] += ndesc
        return q

    xcat = nc.dram_tensor("xcat", [n_nodes, D2], BF16, kind="ExternalInput")
    xneg = nc.dram_tensor("xneg", [n_nodes, D2], BF16, kind="ExternalInput")
    # per-edge relation rows [r_re | -r_im] pre-ordered on host into stream
    # order: rc_seq[p, h, :] is the row for stream position h*128+p. Loaded
    # with one sequential HWDGE DMA per batch (no gather descriptors).
    rc_seq = nc.dram_tensor(
        "rc_seq", [P, e_pad // 128, D2], BF16, kind="ExternalInput"
    )
    idx_s = nc.dram_tensor("idx_s", [P, e_pad // 16], i16, kind="ExternalInput")
    idx_o = nc.dram_tensor("idx_o", [P, e_pad // 16], i16, kind="ExternalInput")
    out = nc.dram_tensor("out", [P, e_pad // 128], f32, kind="ExternalOutput")

    with ExitStack() as ctx:
        tc = ctx.enter_context(tile.TileContext(nc))
        ipool = ctx.enter_context(tc.tile_pool(name="idx", bufs=3))
        gpool = ctx.enter_context(tc.tile_pool(name="gath", bufs=2))
        tpool = ctx.enter_context(tc.tile_pool(name="tmp", bufs=1))
        spool = ctx.enter_context(tc.tile_pool(name="scores", bufs=1))

        scores = spool.tile([P, e_pad // 128], f32)

        for pos, blen, segs in batches:
            g = blen // 128
            cols = blen // 16
            it_s = ipool.tile([P, cols], i16, tag="it_s")
            it_o = ipool.tile([P, cols], i16, tag="it_o")
            nc.sync.dma_start(it_s[:], idx_s[:, pos // 16 : pos // 16 + cols])
            nc.sync.dma_start(it_o[:], idx_o[:, pos // 16 : pos // 16 + cols])

            S = gpool.tile([P, g, D2], BF16, tag="S")
            O = gpool.tile([P, g, D2], BF16, tag="O")
            RC = gpool.tile([P, g, D2], BF16, tag="RC")
            def emit_gather(T, table, it, g0, c0, n):
                # split into two half-gathers on separate queues for better
                # desc-gen concurrency across the 4 SWDGE core pairs
                n1 = ((n // 128 + 1) // 2) * 128 if n >= 256 else n
                for (a, m) in ((0, n1), (n1, n - n1)):
                    if m <= 0:
                        continue
                    nc.gpsimd.dma_gather(
                        T[:, g0 + a // 128 : g0 + (a + m) // 128, :], table,
                        it[:, c0 + a // 16 : c0 + (a + m) // 16], m, m, D2,
                        single_packet=SINGLE_PACKET, queue_num=next_q(m),
                    )

            for (L, n, cs, co) in segs:
                g0 = (L - pos) // 128
                c0 = (L - pos) // 16
                sl_s = xcat[cs * CHUNK : min((cs + 1) * CHUNK, n_nodes), :]
                sl_o = xneg[co * CHUNK : min((co + 1) * CHUNK, n_nodes), :]
                emit_gather(S, sl_s, it_s, g0, c0, n)
                emit_gather(O, sl_o, it_o, g0, c0, n)
            nc.sync.dma_start(
                RC[:, :g, :], rc_seq[:, pos // 128 : pos // 128 + g, :]
            )

            # PQ[:, :, 0:128]   = S * O           -> [s_re*o_re | -s_im*o_im]
            # PQ[:, :, 128:192] = S_hi * O_lo     ->  s_im*o_re
            # PQ[:, :, 192:256] = S_lo * O_hi     -> -s_re*o_im
            PQ = tpool.tile([P, g, 256], BF16, tag="PQ")
            nc.vector.tensor_tensor(
                out=PQ[:, :, 0:128], in0=S[:, :, :], in1=O[:, :, :],
                op=mybir.AluOpType.mult,
            )
            nc.vector.tensor_tensor(
                out=PQ[:, :, 128:192], in0=S[:, :, 64:128], in1=O[:, :, 0:64],
                op=mybir.AluOpType.mult,
            )
            nc.vector.tensor_tensor(
                out=PQ[:, :, 192:256], in0=S[:, :, 0:64], in1=O[:, :, 64:128],
                op=mybir.AluOpType.mult,
            )
            # A = s_re*o_re - s_im*o_im = PQ[0:64] + PQ[64:128]      (add)
            # B = s_im*o_re + s_re*o_im = PQ[128:192] - PQ[192:256]  (subtract)
            AB = tpool.tile([P, g, D2], BF16, tag="AB")
            nc.vector.tensor_tensor(
                out=AB[:, :, 0:64], in0=PQ[:, :, 0:64], in1=PQ[:, :, 64:128],
                op=mybir.AluOpType.add,
            )
            nc.vector.tensor_tensor(
                out=AB[:, :, 64:128], in0=PQ[:, :, 128:192], in1=PQ[:, :, 192:256],
                op=mybir.AluOpType.subtract,
            )
            # W = AB * RC, rc rows = [r_re | -r_im]; overwrite PQ's P-half as scratch
            nc.vector.tensor_tensor(
                out=PQ[:, :, 0:128], in0=AB[:, :, :], in1=RC[:, :g, :],
                op=mybir.AluOpType.mult,
            )
            nc.vector.tensor_reduce(
                out=scores[:, pos // 128 : pos // 128 + g],
                in_=PQ[:, :, 0:128],
                axis=mybir.AxisListType.X,
                op=mybir.AluOpType.add,
            )

        nc.sync.dma_start(out[:], scores[:])
    nc.compile()
    return nc, batches


def kernel(x_re, x_im, R_re, R_im, edge_index, edge_type):
    x_re = np.asarray(x_re, dtype=np.float32)
    x_im = np.asarray(x_im, dtype=np.float32)
    R_re = np.asarray(R_re, dtype=np.float32)
    R_im = np.asarray(R_im, dtype=np.float32)
    src = np.asarray(edge_index[0], dtype=np.int64)
    dst = np.asarray(edge_index[1], dtype=np.int64)
    typ = np.asarray(edge_type, dtype=np.int64)

    n_nodes, d = x_re.shape
    n_rel = R_re.shape[0]
    n_edges = src.shape[0]
    assert d * 2 == D2
    n_chunks = (n_nodes + CHUNK - 1) // CHUNK

    xcat = np.concatenate([x_re, x_im], axis=1).astype(NP_BF16)
    xneg = np.concatenate([x_re, -x_im], axis=1).astype(NP_BF16)
    rcat16 = np.concatenate([R_re, -R_im], axis=1).astype(NP_BF16)

    # ---- deal edges to cores per (src_chunk, dst_chunk) bin ----
    # Round-robin within each bin equalizes per-core bin counts (spread <= 1),
    # minimizing the shared bin capacities and thus padded descriptors.
    n_bins = n_chunks * n_chunks
    bin_id = (src // CHUNK) * n_chunks + (dst // CHUNK)
    order = np.argsort(bin_id, kind="stable")
    counts = np.bincount(bin_id, minlength=n_bins)
    ends = np.cumsum(counts)
    rank_in_bin = np.empty(n_edges, dtype=np.int64)
    rank_in_bin[order] = np.arange(n_edges) - np.concatenate([[0], ends])[bin_id[order]]
    core_of = rank_in_bin % N_CORES
    pos_in_bin = rank_in_bin // N_CORES

    per_core_max = (counts + N_CORES - 1) // N_CORES
    bin_caps = ((per_core_max + 255) // 256 * 256).astype(np.int64)
    e_pad = int(bin_caps.sum())
    bin_starts = np.concatenate([[0], np.cumsum(bin_caps)])
    stream_pos = bin_starts[bin_id] + pos_in_bin  # per-edge slot in its core's stream

    key = (n_nodes, n_rel, e_pad, tuple(bin_caps.tolist()))
    if key not in _cache:
        _cache[key] = _build(n_nodes, n_rel, e_pad, bin_caps, n_chunks)
    nc, _batches = _cache[key]

    # ---- build per-core padded index streams ----
    def wrap16(a):
        w = a.reshape(-1, 16).T  # [16, len/16]
        return np.tile(w, (8, 1)).copy()

    in_maps = []
    for c in range(N_CORES):
        m = core_of == c
        sp = stream_pos[m]
        # pad slots default to chunk-local row 0 / relation 0 (always valid)
        ls = np.zeros(e_pad, dtype=np.int16)
        lo_ = np.zeros(e_pad, dtype=np.int16)
        lr = np.zeros(e_pad, dtype=np.int16)
        ls[sp] = (src[m] % CHUNK).astype(np.int16)
        lo_[sp] = (dst[m] % CHUNK).astype(np.int16)
        lr[sp] = typ[m].astype(np.int16)
        # relation rows in stream order: [128, e_pad//128, D2]
        rc_core = rcat16[lr].reshape(e_pad // 128, P, D2).transpose(1, 0, 2).copy()
        in_maps.append(
            {
                "xcat": xcat,
                "xneg": xneg,
                "rc_seq": rc_core,
                "idx_s": wrap16(ls),
                "idx_o": wrap16(lo_),
            }
        )

    res = run_bass_kernel_spmd(nc, in_maps, core_ids=list(range(N_CORES)))
    global _last_results
    _last_results = res

    # ---- unpermute: stream position i -> out[i % 128, i // 128] ----
    scores = np.empty(n_edges, dtype=np.float32)
    for c in range(N_CORES):
        grid = res.results[c]["out"]  # [128, e_pad//128]
        stream = grid.T.reshape(-1)  # stream[i] = score of stream position i
        m = core_of == c
        scores[m] = stream[stream_pos[m]]
    return scores


# revision 16
# speedup vs baseline: 3.9586x; 1.0916x over previous
"""ComplEx decoder edge scoring on 8 Trainium2 NeuronCores.

score[e] = sum_d Re( s_e * r_e * o_e )  for complex embeddings
         = sum_d [ r_re*(s_re*o_re - s_im*o_im) - r_im*(s_re*o_im + s_im*o_re) ]

Strategy (pure edge parallelism, hint-compliant):
  - Edges dealt round-robin to the 8 cores within each (src_chunk, dst_chunk)
    bin; node/relation tables replicated per core. 32768-row node chunks keep
    chunk-local row ids inside the int16 index format of the SWDGE dma_gather
    instruction. Bin capacities are shared across cores (rounded up to 256)
    so a single SPMD NEFF serves all 8 cores.
  - Device loop per 8192-edge batch: three dma_gather streams pull per-edge
    rows (s 256B from bf16 [x_re|x_im], o 256B from bf16 [x_re|-x_im], and
    relation rows 512B from a 100x100 paired bf16 table [rc_t1|rc_t2] so one
    descriptor serves two edges), then DVE elementwise complex product +
    weighting + free-dim reduction produces one f32 score per edge.
  - Gathers are spread across all 4 SWDGE queues (greedy balance), so
    descriptor generation runs on all 8 GPSIMD Q7 cores instead of 2.
  - Host side only slices/sorts indices and un-permutes the scores.
"""

import os
import numpy as np
import ml_dtypes
from contextlib import ExitStack

import concourse.tile as tile
from concourse import bacc, mybir
from concourse.bass_utils import run_bass_kernel_spmd

N_CORES = 8
CHUNK = 32768          # node-table chunk rows (int16 index limit)
BATCH = 8192           # edges per compute batch
P = 128                # SBUF partitions
D2 = 128               # concat feature dim (2 * D)
N_QUEUES = int(os.environ.get("KQ", "4"))      # SWDGE queues (desc-gen core pairs)
SINGLE_PACKET = os.environ.get("KSP", "0") == "1"

BF16 = mybir.dt.bfloat16
NP_BF16 = ml_dtypes.bfloat16

_cache = {}
_last_results = None


def _build(n_nodes, n_rel, e_pad, bin_caps, n_chunks):
    """Compile the SPMD kernel for the given static layout."""
    f32 = mybir.dt.float32
    i16 = mybir.dt.int16

    # batch boundaries and per-batch gather segments (cut at bin boundaries)
    bin_starts = np.concatenate([[0], np.cumsum(bin_caps)])
    batches = []
    pos = 0
    while pos < e_pad:
        blen = min(BATCH, e_pad - pos)
        segs = []
        for b in range(len(bin_caps)):
            lo = max(pos, int(bin_starts[b]))
            hi = min(pos + blen, int(bin_starts[b + 1]))
            if lo < hi:
                segs.append((lo, hi - lo, b // n_chunks, b % n_chunks))
        batches.append((pos, blen, segs))
        pos += blen

    nc = bacc.Bacc(
        "TRN2",
        num_swdge_queues=N_QUEUES,
        dynamic_dma_scratch_size=int(os.environ.get("KSCRATCH", "32768")),
    )
    qload = [0] * N_QUEUES

    def next_q(ndesc):
        q = qload.index(min(qload))
        qload[q] += ndesc
        return q

    xcat = nc.dram_tensor("xcat", [n_nodes, D2], BF16, kind="ExternalInput")
    xneg = nc.dram_tensor("xneg", [n_nodes, D2], BF16, kind="ExternalInput")
    # per-edge relation rows [r_re | -r_im] pre-ordered on host into stream
    # order: rc_seq[p, h, :] is the row for stream position h*128+p. Loaded
    # with one sequential HWDGE DMA per batch (no gather descriptors).
    rc_seq = nc.dram_tensor(
        "rc_seq", [P, e_pad // 128, D2], BF16, kind="ExternalInput"
    )
    idx_s = nc.dram_tensor("idx_s", [P, e_pad // 16], i16, kind="ExternalInput")
    idx_o = nc.dram_tensor("idx_o", [P, e_pad // 16], i16, kind="ExternalInput")
    out = nc.dram_tensor("out", [P, e_pad // 128], f32, kind="ExternalOutput")

    with ExitStack() as ctx:
        tc = ctx.enter_context(tile.TileContext(nc))
        ipool = ctx.enter_context(tc.tile_pool(name="idx", bufs=3))
        gpool = ctx.enter_context(tc.tile_pool(name="gath", bufs=2))
        tpool = ctx.enter_context(tc.tile_pool(name="tmp", bufs=1))
        spool = ctx.enter_context(tc.tile_pool(name="scores", bufs=1))

        scores = spool.tile([P, e_pad // 128], f32)

        for pos, blen, segs in batches:
            g = blen // 128
            cols = blen // 16
            it_s = ipool.tile([P, cols], i16, tag="it_s")
            it_o = ipool.tile([P, cols], i16, tag="it_o")
            nc.sync.dma_start(it_s[:], idx_s[:, pos // 16 : pos // 16 + cols])
            nc.sync.dma_start(it_o[:], idx_o[:, pos // 16 : pos // 16 + cols])

            S = gpool.tile([P, g, D2], BF16, tag="S")
            O = gpool.tile([P, g, D2], BF16, tag="O")
            RC = gpool.tile([P, g, D2], BF16, tag="RC")
            def emit_gather(T, table, it, g0, c0, n):
                # split into pieces on separate queues: desc-gen runs on all
                # 4 SWDGE core pairs, and <=2048-idx gathers (128 descs per
                # SDMA engine) stay well inside the 512-desc rings
                piece = 2048
                a = 0
                while a < n:
                    m = min(piece, n - a)
                    nc.gpsimd.dma_gather(
                        T[:, g0 + a // 128 : g0 + (a + m) // 128, :], table,
                        it[:, c0 + a // 16 : c0 + (a + m) // 16], m, m, D2,
                        single_packet=SINGLE_PACKET, queue_num=next_q(m),
                    )
                    a += m

            for (L, n, cs, co) in segs:
                g0 = (L - pos) // 128
                c0 = (L - pos) // 16
                sl_s = xcat[cs * CHUNK : min((cs + 1) * CHUNK, n_nodes), :]
                sl_o = xneg[co * CHUNK : min((co + 1) * CHUNK, n_nodes), :]
                emit_gather(S, sl_s, it_s, g0, c0, n)
                emit_gather(O, sl_o, it_o, g0, c0, n)
            nc.sync.dma_start(
                RC[:, :g, :], rc_seq[:, pos // 128 : pos // 128 + g, :]
            )

            # all intermediates are contiguous 128-wide bf16 tiles so DVE can
            # engage 2x_1p (needs step_x=1, 2-byte dtypes, 4B alignment)
            # PQ1 = S * O          -> [s_re*o_re | -s_im*o_im]
            # CR  = [S_hi*O_lo | S_lo*O_hi] -> [s_im*o_re | -s_re*o_im]
            PQ1 = tpool.tile([P, g, D2], BF16, tag="PQ1")
            CR = tpool.tile([P, g, D2], BF16, tag="CR")
            nc.vector.tensor_tensor(
                out=PQ1[:, :, :], in0=S[:, :, :], in1=O[:, :, :],
                op=mybir.AluOpType.mult,
            )
            nc.vector.tensor_tensor(
                out=CR[:, :, 0:64], in0=S[:, :, 64:128], in1=O[:, :, 0:64],
                op=mybir.AluOpType.mult,
            )
            nc.vector.tensor_tensor(
                out=CR[:, :, 64:128], in0=S[:, :, 0:64], in1=O[:, :, 64:128],
                op=mybir.AluOpType.mult,
            )
            # A = s_re*o_re - s_im*o_im = PQ1[0:64] + PQ1[64:128]  (add)
            # B = s_im*o_re + s_re*o_im = CR[0:64] - CR[64:128]    (subtract)
            AB = tpool.tile([P, g, D2], BF16, tag="AB")
            nc.vector.tensor_tensor(
                out=AB[:, :, 0:64], in0=PQ1[:, :, 0:64], in1=PQ1[:, :, 64:128],
                op=mybir.AluOpType.add,
            )
            nc.vector.tensor_tensor(
                out=AB[:, :, 64:128], in0=CR[:, :, 0:64], in1=CR[:, :, 64:128],
                op=mybir.AluOpType.subtract,
            )
            # W = AB * RC, rc rows = [r_re | -r_im]; reuse PQ1 as scratch
            nc.vector.tensor_tensor(
                out=PQ1[:, :, :], in0=AB[:, :, :], in1=RC[:, :g, :],
                op=mybir.AluOpType.mult,
            )
            nc.vector.tensor_reduce(
                out=scores[:, pos // 128 : pos // 128 + g],
                in_=PQ1[:, :, :],
                axis=mybir.AxisListType.X,
                op=mybir.AluOpType.add,
            )

        nc.sync.dma_start(out[:], scores[:])
    nc.compile()
    return nc, batches


def kernel(x_re, x_im, R_re, R_im, edge_index, edge_type):
    x_re = np.asarray(x_re, dtype=np.float32)
    x_im = np.asarray(x_im, dtype=np.float32)
    R_re = np.asarray(R_re, dtype=np.float32)
    R_im = np.asarray(R_im, dtype=np.float32)
    src = np.asarray(edge_index[0], dtype=np.int64)
    dst = np.asarray(edge_index[1], dtype=np.int64)
    typ = np.asarray(edge_type, dtype=np.int64)

    n_nodes, d = x_re.shape
    n_rel = R_re.shape[0]
    n_edges = src.shape[0]
    assert d * 2 == D2
    n_chunks = (n_nodes + CHUNK - 1) // CHUNK

    xcat = np.concatenate([x_re, x_im], axis=1).astype(NP_BF16)
    xneg = np.concatenate([x_re, -x_im], axis=1).astype(NP_BF16)
    rcat16 = np.concatenate([R_re, -R_im], axis=1).astype(NP_BF16)

    # ---- deal edges to cores per (src_chunk, dst_chunk) bin ----
    # Round-robin within each bin equalizes per-core bin counts (spread <= 1),
    # minimizing the shared bin capacities and thus padded descriptors.
    n_bins = n_chunks * n_chunks
    bin_id = (src // CHUNK) * n_chunks + (dst // CHUNK)
    order = np.argsort(bin_id, kind="stable")
    counts = np.bincount(bin_id, minlength=n_bins)
    ends = np.cumsum(counts)
    rank_in_bin = np.empty(n_edges, dtype=np.int64)
    rank_in_bin[order] = np.arange(n_edges) - np.concatenate([[0], ends])[bin_id[order]]
    core_of = rank_in_bin % N_CORES
    pos_in_bin = rank_in_bin // N_CORES

    per_core_max = (counts + N_CORES - 1) // N_CORES
    bin_caps = ((per_core_max + 255) // 256 * 256).astype(np.int64)
    e_pad = int(bin_caps.sum())
    bin_starts = np.concatenate([[0], np.cumsum(bin_caps)])
    stream_pos = bin_starts[bin_id] + pos_in_bin  # per-edge slot in its core's stream

    key = (n_nodes, n_rel, e_pad, tuple(bin_caps.tolist()))
    if key not in _cache:
        _cache[key] = _build(n_nodes, n_rel, e_pad, bin_caps, n_chunks)
    nc, _batches = _cache[key]

    # ---- build per-core padded index streams ----
    def wrap16(a):
        w = a.reshape(-1, 16).T  # [16, len/16]
        return np.tile(w, (8, 1)).copy()

    in_maps = []
    for c in range(N_CORES):
        m = core_of == c
        sp = stream_pos[m]
        # pad slots default to chunk-local row 0 / relation 0 (always valid)
        ls = np.zeros(e_pad, dtype=np.int16)
        lo_ = np.zeros(e_pad, dtype=np.int16)
        lr = np.zeros(e_pad, dtype=np.int16)
        ls[sp] = (src[m] % CHUNK).astype(np.int16)
        lo_[sp] = (dst[m] % CHUNK).astype(np.int16)
        lr[sp] = typ[m].astype(np.int16)
        # relation rows in stream order: [128, e_pad//128, D2]
        rc_core = rcat16[lr].reshape(e_pad // 128, P, D2).transpose(1, 0, 2).copy()
        in_maps.append(
            {
                "xcat": xcat,
                "xneg": xneg,
                "rc_seq": rc_core,
                "idx_s": wrap16(ls),
                "idx_o": wrap16(lo_),
            }
        )

    res = run_bass_kernel_spmd(nc, in_maps, core_ids=list(range(N_CORES)))
    global _last_results
    _last_results = res

    # ---- unpermute: stream position i -> out[i % 128, i // 128] ----
    scores = np.empty(n_edges, dtype=np.float32)
    for c in range(N_CORES):
        grid = res.results[c]["out"]  # [128, e_pad//128]
        stream = grid.T.reshape(-1)  # stream[i] = score of stream position i
        m = core_of == c
        scores[m] = stream[stream_pos[m]]
    return scores


# revision 21
# speedup vs baseline: 4.2479x; 1.0731x over previous
"""ComplEx decoder edge scoring on 8 Trainium2 NeuronCores.

score[e] = sum_d Re( s_e * r_e * o_e )  for complex embeddings
         = sum_d [ r_re*(s_re*o_re - s_im*o_im) - r_im*(s_re*o_im + s_im*o_re) ]

Strategy (pure edge parallelism, hint-compliant):
  - Edges dealt round-robin to the 8 cores within each (src_chunk, dst_chunk)
    bin; node table replicated per core. 32768-row node chunks keep
    chunk-local row ids inside the int16 index format of the SWDGE dma_gather
    instruction. Bin capacities are shared across cores (rounded up to 256)
    so a single SPMD NEFF serves all 8 cores.
  - Device loop per 8192-edge batch: two dma_gather streams pull per-edge
    node rows (s 256B from bf16 [x_re|x_im], o 256B from bf16 [x_re|-x_im]);
    per-edge relation rows are pre-ordered on the host into stream order and
    loaded with one sequential HWDGE DMA per batch (no gather descriptors).
    DVE elementwise complex product + relation weighting + free-dim
    reduction produce one f32 score per edge.
  - Descriptor generation is the machine bottleneck (SWDGE Q7 software
    loop). Three measures attack it: (1) 4 SWDGE queues run desc-gen on 4
    GPSIMD Q7 core pairs instead of 1 (greedy load balance across queues);
    (2) 32KB dynamic-DMA scratch gives 512-descriptor rings per SDMA
    engine so gathers do not stall on ring drain; (3) gathers are emitted
    as 2048-index pieces so pieces overlap across queues.
  - bf16 tables/compute halve DMA bytes and DVE time (rel err ~5e-3,
    tolerance 2e-2). Host side only slices/sorts indices, pre-orders
    relation rows, and un-permutes the scores.

Measured on the 8-core SPMD harness: 2628908 ns (baseline f32, 1 queue)
-> 611923 ns (this version).
"""

import os
import numpy as np
import ml_dtypes
from contextlib import ExitStack

import concourse.tile as tile
from concourse import bacc, mybir
from concourse.bass_utils import run_bass_kernel_spmd

N_CORES = 8
CHUNK = 32768          # node-table chunk rows (int16 index limit)
BATCH = 8192           # edges per compute batch
P = 128                # SBUF partitions
D2 = 128               # concat feature dim (2 * D)
N_QUEUES = 4          # SWDGE queues: desc-gen runs on 4 GPSIMD Q7 core pairs
SINGLE_PACKET = False  # True wedges the device (coalesced packet starves runtime)
DMA_SCRATCH = 32768   # 512-desc rings per SDMA engine: gathers fit without stalls
GATHER_PIECE = 2048   # idxs per dma_gather: small pieces overlap across queues

BF16 = mybir.dt.bfloat16
NP_BF16 = ml_dtypes.bfloat16

_cache = {}
_last_results = None


def _build(n_nodes, n_rel, e_pad, bin_caps, n_chunks):
    """Compile the SPMD kernel for the given static layout."""
    f32 = mybir.dt.float32
    i16 = mybir.dt.int16

    # batch boundaries and per-batch gather segments (cut at bin boundaries)
    bin_starts = np.concatenate([[0], np.cumsum(bin_caps)])
    batches = []
    pos = 0
    while pos < e_pad:
        blen = min(BATCH, e_pad - pos)
        segs = []
        for b in range(len(bin_caps)):
            lo = max(pos, int(bin_starts[b]))
            hi = min(pos + blen, int(bin_starts[b + 1]))
            if lo < hi:
                segs.append((lo, hi - lo, b // n_chunks, b % n_chunks))
        batches.append((pos, blen, segs))
        pos += blen

    nc = bacc.Bacc(
        "TRN2",
        num_swdge_queues=N_QUEUES,
        dynamic_dma_scratch_size=DMA_SCRATCH,
    )
    qload = [0] * N_QUEUES

    def next_q(ndesc):
        q = qload.index(min(qload))
        qload[q] += ndesc
        return q

    xcat = nc.dram_tensor("xcat", [n_nodes, D2], BF16, kind="ExternalInput")
    xneg = nc.dram_tensor("xneg", [n_nodes, D2], BF16, kind="ExternalInput")
    # per-edge relation rows [r_re | -r_im] pre-ordered on host into stream
    # order: rc_seq[p, h, :] is the row for stream position h*128+p. Loaded
    # with one sequential HWDGE DMA per batch (no gather descriptors).
    rc_seq = nc.dram_tensor(
        "rc_seq", [P, e_pad // 128, D2], BF16, kind="ExternalInput"
    )
    idx_s = nc.dram_tensor("idx_s", [P, e_pad // 16], i16, kind="ExternalInput")
    idx_o = nc.dram_tensor("idx_o", [P, e_pad // 16], i16, kind="ExternalInput")
    out = nc.dram_tensor("out", [P, e_pad // 128], f32, kind="ExternalOutput")

    with ExitStack() as ctx:
        tc = ctx.enter_context(tile.TileContext(nc))
        ipool = ctx.enter_context(tc.tile_pool(name="idx", bufs=3))
        gpool = ctx.enter_context(tc.tile_pool(name="gath", bufs=2))
        tpool = ctx.enter_context(tc.tile_pool(name="tmp", bufs=1))
        spool = ctx.enter_context(tc.tile_pool(name="scores", bufs=1))

        scores = spool.tile([P, e_pad // 128], f32)

        for pos, blen, segs in batches:
            g = blen // 128
            cols = blen // 16
            it_s = ipool.tile([P, cols], i16, tag="it_s")
            it_o = ipool.tile([P, cols], i16, tag="it_o")
            nc.sync.dma_start(it_s[:], idx_s[:, pos // 16 : pos // 16 + cols])
            nc.sync.dma_start(it_o[:], idx_o[:, pos // 16 : pos // 16 + cols])

            S = gpool.tile([P, g, D2], BF16, tag="S")
            O = gpool.tile([P, g, D2], BF16, tag="O")
            RC = gpool.tile([P, g, D2], BF16, tag="RC")
            def emit_gather(T, table, it, g0, c0, n):
                # split into pieces on separate queues: desc-gen runs on all
                # 4 SWDGE core pairs, and <=2048-idx gathers (128 descs per
                # SDMA engine) stay well inside the 512-desc rings
                a = 0
                while a < n:
                    m = min(GATHER_PIECE, n - a)
                    nc.gpsimd.dma_gather(
                        T[:, g0 + a // 128 : g0 + (a + m) // 128, :], table,
                        it[:, c0 + a // 16 : c0 + (a + m) // 16], m, m, D2,
                        single_packet=SINGLE_PACKET, queue_num=next_q(m),
                    )
                    a += m

            for (L, n, cs, co) in segs:
                g0 = (L - pos) // 128
                c0 = (L - pos) // 16
                sl_s = xcat[cs * CHUNK : min((cs + 1) * CHUNK, n_nodes), :]
                sl_o = xneg[co * CHUNK : min((co + 1) * CHUNK, n_nodes), :]
                emit_gather(S, sl_s, it_s, g0, c0, n)
                emit_gather(O, sl_o, it_o, g0, c0, n)
            nc.sync.dma_start(
                RC[:, :g, :], rc_seq[:, pos // 128 : pos // 128 + g, :]
            )

            # PQ[:, :, 0:128]   = S * O           -> [s_re*o_re | -s_im*o_im]
            # PQ[:, :, 128:192] = S_hi * O_lo     ->  s_im*o_re
            # PQ[:, :, 192:256] = S_lo * O_hi     -> -s_re*o_im
            PQ = tpool.tile([P, g, 256], BF16, tag="PQ")
            nc.vector.tensor_tensor(
                out=PQ[:, :, 0:128], in0=S[:, :, :], in1=O[:, :, :],
                op=mybir.AluOpType.mult,
            )
            nc.vector.tensor_tensor(
                out=PQ[:, :, 128:192], in0=S[:, :, 64:128], in1=O[:, :, 0:64],
                op=mybir.AluOpType.mult,
            )
            nc.vector.tensor_tensor(
                out=PQ[:, :, 192:256], in0=S[:, :, 0:64], in1=O[:, :, 64:128],
                op=mybir.AluOpType.mult,
            )
            # A = s_re*o_re - s_im*o_im = PQ[0:64] + PQ[64:128]      (add)
            # B = s_im*o_re + s_re*o_im = PQ[128:192] - PQ[192:256]  (subtract)
            AB = tpool.tile([P, g, D2], BF16, tag="AB")
            nc.vector.tensor_tensor(
                out=AB[:, :, 0:64], in0=PQ[:, :, 0:64], in1=PQ[:, :, 64:128],
                op=mybir.AluOpType.add,
            )
            nc.vector.tensor_tensor(
                out=AB[:, :, 64:128], in0=PQ[:, :, 128:192], in1=PQ[:, :, 192:256],
                op=mybir.AluOpType.subtract,
            )
            # W = AB * RC, rc rows = [r_re | -r_im]; overwrite PQ's P-half as scratch
            nc.vector.tensor_tensor(
                out=PQ[:, :, 0:128], in0=AB[:, :, :], in1=RC[:, :g, :],
                op=mybir.AluOpType.mult,
            )
            nc.vector.tensor_reduce(
                out=scores[:, pos // 128 : pos // 128 + g],
                in_=PQ[:, :, 0:128],
                axis=mybir.AxisListType.X,
                op=mybir.AluOpType.add,
            )

        nc.sync.dma_start(out[:], scores[:])
    nc.compile()
    return nc, batches


def kernel(x_re, x_im, R_re, R_im, edge_index, edge_type):
    x_re = np.asarray(x_re, dtype=np.float32)
    x_im = np.asarray(x_im, dtype=np.float32)
    R_re = np.asarray(R_re, dtype=np.float32)
    R_im = np.asarray(R_im, dtype=np.float32)
    src = np.asarray(edge_index[0], dtype=np.int64)
    dst = np.asarray(edge_index[1], dtype=np.int64)
    typ = np.asarray(edge_type, dtype=np.int64)

    n_nodes, d = x_re.shape
    n_rel = R_re.shape[0]
    n_edges = src.shape[0]
    assert d * 2 == D2
    n_chunks = (n_nodes + CHUNK - 1) // CHUNK

    xcat = np.concatenate([x_re, x_im], axis=1).astype(NP_BF16)
    xneg = np.concatenate([x_re, -x_im], axis=1).astype(NP_BF16)
    rcat16 = np.concatenate([R_re, -R_im], axis=1).astype(NP_BF16)

    # ---- deal edges to cores per (src_chunk, dst_chunk) bin ----
    # Round-robin within each bin equalizes per-core bin counts (spread <= 1),
    # minimizing the shared bin capacities and thus padded descriptors.
    n_bins = n_chunks * n_chunks
    bin_id = (src // CHUNK) * n_chunks + (dst // CHUNK)
    order = np.argsort(bin_id, kind="stable")
    counts = np.bincount(bin_id, minlength=n_bins)
    ends = np.cumsum(counts)
    rank_in_bin = np.empty(n_edges, dtype=np.int64)
    rank_in_bin[order] = np.arange(n_edges) - np.concatenate([[0], ends])[bin_id[order]]
    core_of = rank_in_bin % N_CORES
    pos_in_bin = rank_in_bin // N_CORES

    per_core_max = (counts + N_CORES - 1) // N_CORES
    bin_caps = ((per_core_max + 255) // 256 * 256).astype(np.int64)
    e_pad = int(bin_caps.sum())
    bin_starts = np.concatenate([[0], np.cumsum(bin_caps)])
    stream_pos = bin_starts[bin_id] + pos_in_bin  # per-edge slot in its core's stream

    key = (n_nodes, n_rel, e_pad, tuple(bin_caps.tolist()))
    if key not in _cache:
        _cache[key] = _build(n_nodes, n_rel, e_pad, bin_caps, n_chunks)
    nc, _batches = _cache[key]

    # ---- build per-core padded index streams ----
    def wrap16(a):
        w = a.reshape(-1, 16).T  # [16, len/16]
        return np.tile(w, (8, 1)).copy()

    in_maps = []
    for c in range(N_CORES):
        m = core_of == c
        sp = stream_pos[m]
        # pad slots default to chunk-local row 0 / relation 0 (always valid)
        ls = np.zeros(e_pad, dtype=np.int16)
        lo_ = np.zeros(e_pad, dtype=np.int16)
        lr = np.zeros(e_pad, dtype=np.int16)
        ls[sp] = (src[m] % CHUNK).astype(np.int16)
        lo_[sp] = (dst[m] % CHUNK).astype(np.int16)
        lr[sp] = typ[m].astype(np.int16)
        # relation rows in stream order: [128, e_pad//128, D2]
        rc_core = rcat16[lr].reshape(e_pad // 128, P, D2).transpose(1, 0, 2).copy()
        in_maps.append(
            {
                "xcat": xcat,
                "xneg": xneg,
                "rc_seq": rc_core,
                "idx_s": wrap16(ls),
                "idx_o": wrap16(lo_),
            }
        )

    res = run_bass_kernel_spmd(nc, in_maps, core_ids=list(range(N_CORES)))
    global _last_results
    _last_results = res

    # ---- unpermute: stream position i -> out[i % 128, i // 128] ----
    scores = np.empty(n_edges, dtype=np.float32)
    for c in range(N_CORES):
        grid = res.results[c]["out"]  # [128, e_pad//128]
        stream = grid.T.reshape(-1)  # stream[i] = score of stream position i
        m = core_of == c
        scores[m] = stream[stream_pos[m]]
    return scores
